# revision 22
# baseline (speedup 1.0000x reference)
"""ALiBi causal multihead attention on 8 TRN2 NeuronCores.

Sharding: tensor-parallel over heads (16 heads -> 2 per core).
  - Column-parallel Wq/Wk/Wv: core c computes projections for output dims
    [c*128, (c+1)*128) (its two heads).
  - Attention fully local per (batch, head).
  - Row-parallel Wo: each core emits a full-shape partial output; the host
    sums the 8 partials (the unshard step) and adds bo.

Device layout choices (no on-device transposes anywhere):
  - Host pre-transposes q/k/v to [B, D, S] bf16 so projections contract over
    D with D on partitions.
  - Q,K are produced transposed: [d', s] (d' on partitions).
  - Scores are computed transposed: [keys, q] = K_T.T @ Q_T.
  - Softmax uses no max-subtraction (scores are bounded, exp is safe); the
    ALiBi bias + causal mask + key padding mask are folded into a single
    host-precomputed multiplicative exp(bias) tensor (masked entries exactly
    0), applied with one vector multiply after exp(scores).
  - V is produced in natural [s, d'] layout with an appended ones column, so
    the P@V matmul (lhsT=V_aug, rhs=P_T) yields O_T[d', q] plus the softmax
    denominator row in one accumulation group.
  - Normalization: reciprocal of the denominator row, broadcast across
    partitions with a tiny ones-column matmul, one vector multiply.
  - Wo needs no transpose either: lhsT=O_2h[d', s-chunk], rhs=Wo_cT.
"""

import numpy as np
import ml_dtypes

B, S, D, H = 4, 1024, 1024, 16
DK = D // H  # 64
N_CORES = 8
HPC = H // N_CORES  # heads per core = 2
DPC = HPC * DK  # 128 output dims per core

BF16 = ml_dtypes.bfloat16

_BUILT = {}
_PATCHED = False


def _patch_tile_drain():
    # kept for compatibility; superseded by _split_sync_waits
    pass


_WAITSPLIT_N = [0]


def _split_sync_waits(nc, limit=1):
    """This walrus build rejects instructions carrying more than ~1 sync
    wait. Strip excess waits onto dedicated same-engine nops spliced
    immediately before the instruction (same sequencer => same semantics)."""
    import concourse.mybir as mybir

    for fn in nc.m.functions:
        for bb in fn.blocks:
            out = []
            changed = False
            for inst in bb.instructions:
                si = inst.sync_info
                if si is not None and si.on_wait and len(si.on_wait) > limit:
                    waits = list(si.on_wait)
                    si.on_wait = waits[:limit]
                    for w in waits[limit:]:
                        _WAITSPLIT_N[0] += 1
                        nop = mybir.InstNoOp(
                            name=f"waitsplit_{_WAITSPLIT_N[0]}",
                            engine=inst.engine,
                            ins=[],
                            outs=[],
                            sync_info=mybir.SyncInfo(on_wait=[w], on_update=[]),
                        )
                        out.append(nop)
                    changed = True
                out.append(inst)
            if changed:
                bb.instructions = out


def _build(nb, split=True):
    """Build the per-core Bass graph. nb = bias batch dim (1 when the key
    padding mask is batch-uniform, else B)."""
    import concourse.bass as bass
    import concourse.mybir as mybir
    from concourse.tile import TileContext

    f32 = mybir.dt.float32
    f32r = mybir.dt.float32r
    bf16 = mybir.dt.bfloat16
    Exp = mybir.ActivationFunctionType.Exp
    Ln = mybir.ActivationFunctionType.Ln

    nc = bass.Bass()

    xq = nc.declare_dram_parameter("xq", [B, D, S], bf16, isOutput=False)
    xk = nc.declare_dram_parameter("xk", [B, D, S], bf16, isOutput=False)
    xv = nc.declare_dram_parameter("xv", [B, D, S], bf16, isOutput=False)
    wq = nc.declare_dram_parameter("wq", [128, D], bf16, isOutput=False)
    wk = nc.declare_dram_parameter("wk", [128, D], bf16, isOutput=False)
    wv = nc.declare_dram_parameter("wv", [128, D], bf16, isOutput=False)
    wo = nc.declare_dram_parameter("wo", [128, D], bf16, isOutput=False)
    bqp = nc.declare_dram_parameter("bq", [128, 1], f32, isOutput=False)
    bkp = nc.declare_dram_parameter("bk", [128, 1], f32, isOutput=False)
    bvp = nc.declare_dram_parameter("bv", [1, 128], f32, isOutput=False)
    eb = nc.declare_dram_parameter(
        "ebias", [nb, HPC, S, S] if nb > 1 else [HPC, S, S], bf16, isOutput=False
    )
    out = nc.declare_dram_parameter("out", [B, S, D], bf16, isOutput=True)

    NQB = 2  # 512-wide query blocks
    QBW = S // NQB
    NKC = S // 128  # 8 key chunks of 128

    def valid_kcs(qb):
        # keys chunk kc is (partially) unmasked for query block qb iff
        # kc*128 <= qb*QBW + QBW - 1
        return [kc for kc in range(NKC) if kc * 128 <= qb * QBW + QBW - 1]

    with TileContext(nc) as tc:
        with (
            tc.tile_pool(name="const", bufs=1) as cpool,
            tc.tile_pool(name="qt", bufs=2) as qt_pool,
            tc.tile_pool(name="kt", bufs=2) as kt_pool,
            tc.tile_pool(name="vsb", bufs=2) as vsb_pool,
            tc.tile_pool(name="o2h", bufs=2) as o2h_pool,
            tc.tile_pool(name="xrhs", bufs=10) as xrhs_pool,
            tc.tile_pool(name="xvrow", bufs=10) as xvrow_pool,
            tc.tile_pool(name="et", bufs=3) as et_pool,
            tc.tile_pool(name="pt", bufs=3) as pt_pool,
            tc.tile_pool(name="rc", bufs=2) as rc_pool,
            tc.tile_pool(name="rb", bufs=2) as rb_pool,
            tc.tile_pool(name="ot", bufs=3) as ot_pool,
            tc.tile_pool(name="psS", bufs=4, space="PSUM") as psS,  # proj/scores
            tc.tile_pool(name="psO", bufs=2, space="PSUM") as psO,  # O_T+denom
            tc.tile_pool(name="psW", bufs=2, space="PSUM") as psW,  # Wo + bcast
        ):
            # ---- constants ----
            wq_sb = cpool.tile([128, D], bf16, tag="wq")
            wk_sb = cpool.tile([128, D], bf16, tag="wk")
            wv_sb = cpool.tile([128, D], bf16, tag="wv")
            wo_sb = cpool.tile([128, D], bf16, tag="wo")
            bq_sb = cpool.tile([128, 1], f32, tag="bq")
            bk_sb = cpool.tile([128, 1], f32, tag="bk")
            bv_sb = cpool.tile([1, 128], f32, tag="bv")
            ones_sb = cpool.tile([1, 128], f32, tag="ones")
            ones_bf = cpool.tile([1, 64], bf16, tag="onesbf")
            bvbc_sb = cpool.tile([128, 128], f32, tag="bvbc")
            # resident exp-bias tiles: one contiguous [128, 512] tile per
            # (h, qb, valid kc) so the DVE multiply sees unit-pitch operands
            eb_sb = {}
            for bi in range(nb):
                for h in range(HPC):
                    for qb in range(NQB):
                        for kc in valid_kcs(qb):
                            eb_sb[(bi, h, qb, kc)] = cpool.tile(
                                [128, QBW],
                                bf16,
                                tag=f"eb{bi}_{h}_{qb}_{kc}",
                                name=f"eb{bi}_{h}_{qb}_{kc}",
                            )

            nc.sync.dma_start(out=wq_sb[:], in_=wq[:])
            nc.sync.dma_start(out=wk_sb[:], in_=wk[:])
            nc.sync.dma_start(out=wv_sb[:], in_=wv[:])
            nc.sync.dma_start(out=wo_sb[:], in_=wo[:])
            nc.sync.dma_start(out=bq_sb[:], in_=bqp[:])
            nc.sync.dma_start(out=bk_sb[:], in_=bkp[:])
            nc.sync.dma_start(out=bv_sb[:], in_=bvp[:])
            nc.vector.memset(ones_sb[:], 1.0)
            nc.vector.memset(ones_bf[:], 1.0)
            for (bi, h, qb, kc), tile_ in eb_sb.items():
                src = (
                    eb[bi, h, kc * 128 : (kc + 1) * 128, qb * QBW : (qb + 1) * QBW]
                    if nb > 1
                    else eb[h, kc * 128 : (kc + 1) * 128, qb * QBW : (qb + 1) * QBW]
                )
                nc.sync.dma_start(out=tile_[:], in_=src)

            # one-time: broadcast bv row across partitions via PE
            ps_bv = psW.tile([128, 128], f32, tag="pw")
            nc.tensor.matmul(
                ps_bv[:], lhsT=ones_sb[:, :128], rhs=bv_sb[:], start=True, stop=True
            )
            nc.vector.tensor_copy(bvbc_sb[:], ps_bv[:])

            def phaseA(b):
                # ---- phase A: projections ----
                qt = qt_pool.tile([128, S], bf16, tag="qt")
                kt = kt_pool.tile([128, S], bf16, tag="kt")
                for nm, dst, w_sb, b_sb, x in (
                    ("q", qt, wq_sb, bq_sb, xq),
                    ("k", kt, wk_sb, bk_sb, xk),
                ):
                    rows = []
                    for dc in range(8):
                        xt = xrhs_pool.tile(
                            [128, S], bf16, tag="xrhs", name=f"xr{nm}{dc}"
                        )
                        nc.sync.dma_start(
                            out=xt[:], in_=x[b, dc * 128 : (dc + 1) * 128, :]
                        )
                        rows.append(xt)
                    for sc in range(NQB):
                        ps = psS.tile([128, QBW], f32, tag="ps", name="psproj")
                        for dc in range(8):
                            nc.tensor.matmul(
                                ps[:],
                                lhsT=w_sb[:, dc * 128 : (dc + 1) * 128],
                                rhs=rows[dc][:, sc * QBW : (sc + 1) * QBW],
                                start=(dc == 0),
                                stop=(dc == 7),
                            )
                        nc.vector.tensor_scalar_add(
                            out=dst[:, sc * QBW : (sc + 1) * QBW],
                            in0=ps[:],
                            scalar1=b_sb[:],
                        )

                # V projection: natural [s, d'] layout. All 8 xv row tiles
                # stay resident for the batch; each 128-wide column slice of
                # a psum tile is one complete accumulation group (PSUM groups
                # must not interleave within a tile).
                vsb = vsb_pool.tile([128, NKC * 130], bf16, tag="vsb")
                xrows = []
                for dc in range(8):
                    xrow = xvrow_pool.tile(
                        [128, S], bf16, tag="xvrow", name=f"xvrow{dc}"
                    )
                    nc.sync.dma_start(
                        out=xrow[:], in_=xv[b, dc * 128 : (dc + 1) * 128, :]
                    )
                    xrows.append(xrow)

                def v_asm(s8, pv):
                    cg = (s8 % 4) * 128
                    base = s8 * 130
                    nc.vector.tensor_add(
                        vsb[:, base : base + 64],
                        pv[:, cg : cg + 64],
                        bvbc_sb[:, 0:64],
                    )
                    nc.vector.tensor_add(
                        vsb[:, base + 65 : base + 129],
                        pv[:, cg + 64 : cg + 128],
                        bvbc_sb[:, 64:128],
                    )
                    nc.vector.memset(vsb[:, base + 64 : base + 65], 1.0)
                    nc.vector.memset(vsb[:, base + 129 : base + 130], 1.0)

                ps_v = [None, None]
                for g in range(2):
                    ps_v[g] = psS.tile([128, 512], f32, tag="ps", name=f"psv{g}")
                    for s8 in range(g * 4, g * 4 + 4):
                        cg = (s8 % 4) * 128
                        for dc in range(8):
                            nc.tensor.matmul(
                                ps_v[g][:, cg : cg + 128],
                                lhsT=xrows[dc][:, s8 * 128 : (s8 + 1) * 128],
                                rhs=wv_sb[:, dc * 128 : (dc + 1) * 128],
                                start=(dc == 0),
                                stop=(dc == 7),
                            )
                for g in range(2):
                    for s8 in range(g * 4, g * 4 + 4):
                        v_asm(s8, ps_v[g])
                return qt, kt, vsb

            def phaseB(b, qt, kt, vsb):
                # ---- phase B: attention (heads interleaved for PE density) ----
                bi = b if nb > 1 else 0
                o2h = o2h_pool.tile([128, S], bf16, tag="o2h")
                for qb in range(NQB):
                    kcs = valid_kcs(qb)
                    po_h = []
                    for h in range(HPC):
                        po_h.append(
                            psO.tile([65, QBW], f32, tag="po", name=f"po{h}")
                        )
                    # software pipeline: scores/exp/mul run one kc ahead of
                    # the P@V consumer, across both heads (2x independent work)
                    pend = {}
                    steps = [(kc, h) for kc in kcs for h in range(HPC)]
                    LOOK = 2  # in units of (kc, h) steps
                    for i in range(len(steps) + LOOK):
                        if i < len(steps):
                            kc, h = steps[i]
                            ps = psS.tile([128, QBW], f32, tag="ps")
                            nc.tensor.matmul(
                                ps[:],
                                lhsT=kt[
                                    h * 64 : (h + 1) * 64, kc * 128 : (kc + 1) * 128
                                ],
                                rhs=qt[
                                    h * 64 : (h + 1) * 64, qb * QBW : (qb + 1) * QBW
                                ],
                                start=True,
                                stop=True,
                            )
                            et = et_pool.tile([128, QBW], bf16, tag="et")
                            nc.scalar.activation(et[:], ps[:], Exp)
                            pt = pt_pool.tile([128, QBW], bf16, tag="pt")
                            mul_eng = nc.gpsimd if (i % 3 == 2) else nc.vector
                            mul_eng.tensor_mul(
                                pt[:], et[:], eb_sb[(bi, h, qb, kc)][:]
                            )
                            pend[i] = (kc, h, pt)
                        j = i - LOOK
                        if j >= 0:
                            kc, h, pt = pend.pop(j)
                            nc.tensor.matmul(
                                po_h[h][:],
                                lhsT=vsb[
                                    :, kc * 130 + h * 65 : kc * 130 + (h + 1) * 65
                                ],
                                rhs=pt[:],
                                start=(kc == kcs[0]),
                                stop=(kc == kcs[-1]),
                            )
                    # normalize both heads: reciprocal of the denominator
                    # row computed on ScalarE as exp(-ln(d)) (row ops, LUT
                    # accuracy ~1e-4), broadcast across partitions with a
                    # bf16 ones-matmul, then two multiplies.
                    ps_bc = psW.tile([128, QBW], f32, tag="pw")
                    for h in range(HPC):
                        lnt = rc_pool.tile(
                            [1, QBW], f32, tag="lnt", name=f"lnt{h}"
                        )
                        nc.scalar.activation(lnt[:], po_h[h][64:65, :], Ln)
                        db = rc_pool.tile([1, QBW], bf16, tag="rc", name=f"db{h}")
                        nc.scalar.activation(db[:], lnt[:], Exp, scale=-1.0)
                        nc.tensor.matmul(
                            ps_bc[h * 64 : (h + 1) * 64, :],
                            lhsT=ones_bf[:],
                            rhs=db[:],
                            start=True,
                            stop=True,
                        )
                    rb = rb_pool.tile([128, QBW], f32, tag="rb")
                    nc.vector.tensor_copy(rb[:], ps_bc[:])
                    for h in range(HPC):
                        nc.vector.tensor_mul(
                            o2h[h * 64 : (h + 1) * 64, qb * QBW : (qb + 1) * QBW],
                            po_h[h][0:64, :],
                            rb[h * 64 : (h + 1) * 64, :],
                        )

                return o2h

            def phaseC(b, o2h):
                # ---- phase C: output projection (partial over this core's d') ----
                for s8 in range(8):
                    otile = ot_pool.tile([128, S], bf16, tag="ot")
                    for do in range(NQB):
                        pw = psW.tile([128, QBW], f32, tag="pw")
                        nc.tensor.matmul(
                            pw[:],
                            lhsT=o2h[:, s8 * 128 : (s8 + 1) * 128],
                            rhs=wo_sb[:, do * QBW : (do + 1) * QBW],
                            start=True,
                            stop=True,
                        )
                        nc.any.tensor_copy(
                            otile[:, do * QBW : (do + 1) * QBW], pw[:]
                        )
                    nc.sync.dma_start(
                        out=out[b, s8 * 128 : (s8 + 1) * 128, :], in_=otile[:]
                    )

            # pipelined emission: C(b-1) is emitted after A(b) so the PE
            # stream never waits on the normalize chain at phase seams
            o2h_prev = None
            for b in range(B):
                qt, kt, vsb = phaseA(b)
                if o2h_prev is not None:
                    phaseC(b - 1, o2h_prev)
                o2h_prev = phaseB(b, qt, kt, vsb)
            phaseC(B - 1, o2h_prev)
    if split:
        _split_sync_waits(nc)
    return nc


def _get_built(nb):
    if nb not in _BUILT:
        _BUILT[nb] = _build(nb)
    return _BUILT[nb]


def _prepare(inputs):
    query = np.asarray(inputs["query"], np.float32)
    key = np.asarray(inputs["key"], np.float32)
    value = np.asarray(inputs["value"], np.float32)
    alibi = np.asarray(inputs["alibi_bias"], np.float32)
    kpm = np.asarray(inputs["key_padding_mask"])
    Wq = np.asarray(inputs["Wq"], np.float32)
    bq = np.asarray(inputs["bq"], np.float32)
    Wk = np.asarray(inputs["Wk"], np.float32)
    bk = np.asarray(inputs["bk"], np.float32)
    Wv = np.asarray(inputs["Wv"], np.float32)
    bv = np.asarray(inputs["bv"], np.float32)
    Wo = np.asarray(inputs["Wo"], np.float32)

    scale = 1.0 / np.sqrt(np.float32(DK))

    xq = np.ascontiguousarray(query.transpose(0, 2, 1)).astype(BF16)
    xk = np.ascontiguousarray(key.transpose(0, 2, 1)).astype(BF16)
    xv = np.ascontiguousarray(value.transpose(0, 2, 1)).astype(BF16)

    # exp(alibi + causal + padding) — masked entries exactly 0, transposed to
    # [h, key, query] to match the on-device transposed-scores layout.
    ii = np.arange(S)
    causal_ok = ii[None, :] <= ii[:, None]  # [q, k] True where visible
    uniform = bool(np.all(kpm == kpm[0:1]))
    nb = 1 if uniform else B

    def make_ebias(mask_row):
        ok = causal_ok & (~mask_row)[None, :]  # [q, k]
        with np.errstate(over="ignore", under="ignore"):
            e = np.exp(alibi)  # [H, q, k]
        e = np.where(ok[None], e, 0.0).astype(np.float32)
        return np.ascontiguousarray(e.transpose(0, 2, 1)).astype(BF16)  # [H, k, q]

    if uniform:
        ebias_all = make_ebias(np.asarray(kpm[0], bool))  # [H, S, S]
    else:
        ebias_all = np.stack(
            [make_ebias(np.asarray(kpm[b], bool)) for b in range(B)]
        )  # [B, H, S, S]

    in_maps = []
    for c in range(N_CORES):
        lo, hi = c * DPC, (c + 1) * DPC
        wq_c = ((Wq[lo:hi, :] * scale).astype(np.float32)).astype(BF16)
        wk_c = Wk[lo:hi, :].astype(BF16)
        wv_c = Wv[lo:hi, :].astype(BF16)
        # packed[p, dc*128+m] = Wc[m, dc*128+p]
        pack = lambda Wc: np.ascontiguousarray(
            Wc.reshape(128, 8, 128).transpose(2, 1, 0).reshape(128, D)
        )
        wo_c = np.ascontiguousarray(Wo[:, lo:hi].T).astype(BF16)  # [128, D]
        hlo = c * HPC
        ebc = (
            ebias_all[hlo : hlo + HPC]
            if uniform
            else ebias_all[:, hlo : hlo + HPC]
        )
        in_maps.append(
            {
                "xq": xq,
                "xk": xk,
                "xv": xv,
                "wq": pack(wq_c),
                "wk": pack(wk_c),
                "wv": pack(wv_c),
                "wo": wo_c,
                "bq": (bq[lo:hi] * scale).astype(np.float32).reshape(128, 1),
                "bk": bk[lo:hi].astype(np.float32).reshape(128, 1),
                "bv": bv[lo:hi].astype(np.float32).reshape(1, 128),
                "ebias": np.ascontiguousarray(ebc),
            }
        )
    return nb, in_maps


def _run(inputs, trace=False):
    from concourse.bass_utils import run_bass_kernel_spmd

    nb, in_maps = _prepare(inputs)
    nc = _get_built(nb)
    res = run_bass_kernel_spmd(
        nc, in_maps, list(range(N_CORES)), trace=trace
    )
    acc = np.zeros((B, S, D), np.float32)
    for c in range(N_CORES):
        acc += np.asarray(res.results[c]["out"], np.float32)
    acc += np.asarray(inputs["bo"], np.float32)[None, None, :]
    return acc, res


def kernel(**inputs):
    out, _ = _run(inputs)
    return out
